# revision 1
# baseline (speedup 1.0000x reference)
"""CrissCrossAttention Trainium2 kernel (8 NeuronCores, data-parallel).

Problem: B=4, C=256, H=W=128, 4 heads. Per head: cq=8 q/k channels, cv=64
v channels. Row attention (over W per row) + column attention (over H per
column), outputs added with the CCNet spatial-transpose quirk, then
out = gamma*attn + x.

Sharding: 16 (batch, head) pairs over 8 cores -> each core handles
batch b = core//2 and head pair p = core%2 (global heads 2p, 2p+1).
Each core reads x[b] (all 256 channels, needed by the projections) and
produces output channels [128p : 128p+128] of batch b.

v2 structure (vs v1 baseline):
  - qk PSUM evacuation on ACT (Identity + per-partition bias) instead of
    DVE tensor_scalar_add; x f32->bf16 casts alternate DVE/ACT.
  - v bias folded in at evacuation via a host-replicated [128,130] bias
    row and a DVE add (kills the K=1 N=260 bias matmuls).
  - q/k banding SBUF->SBUF DMAs issued incrementally inside phase B
    (per 8-chunk h-slice); qc/kc banding issued at the end of B so it
    overlaps the vTc transpose phase. Kills the ~35us post-B DMA stall.
  - vTc: ones-columns (64/129) memset instead of transposed; PSUM
    evacuations alternate DVE/ACT.
  - Phase C: one [128,4,512] f32 PSUM "mega tile" (4 banks) per (g,d)
    holds energy (hh packed, [0:256]), AV outputs+denoms ([256:386]) and
    the output transpose ([0:64] as bf16, time-multiplexed after EXP
    consumed the energies). bufs=2 -> the whole working set double
    buffers in 8 banks, breaking the energy->EXP->energy serialization.
    One EXP (FD=1024) and one normalize mult (FD=512) per (g,d).
"""

import os
import numpy as np
from contextlib import ExitStack

import concourse.bass as bass
import concourse.bacc as bacc
import concourse.tile as tile
from concourse import mybir
from concourse.masks import make_identity

F32 = mybir.dt.float32
BF16 = mybir.dt.bfloat16

B, C, H, W = 4, 256, 128, 128
PIX = H * W            # 16384
CV = 64                # v channels per head
NCORES = 8
G = 4                  # rows per attention group (= PE row-group packing)
NG = H // G            # 32 groups

AF = mybir.ActivationFunctionType


def build_program():
    nc = bacc.Bacc("TRN2", target_bir_lowering=False, debug=False,
                   num_devices=NCORES)

    x_in = nc.dram_tensor("x_in", [C, PIX], F32, kind="ExternalInput")
    x_res = nc.dram_tensor("x_res", [128, PIX], F32, kind="ExternalInput")
    wqkT = nc.dram_tensor("wqkT", [C, 32], BF16, kind="ExternalInput")
    qk_bias = nc.dram_tensor("qk_bias", [32, 1], F32, kind="ExternalInput")
    wvT = nc.dram_tensor("wvT", [C, 130], BF16, kind="ExternalInput")
    vbias_bc = nc.dram_tensor("vbias_bc", [128, 130], BF16, kind="ExternalInput")
    out = nc.dram_tensor("out", [128, PIX], F32, kind="ExternalOutput")

    with tile.TileContext(nc) as tc, ExitStack() as ctx:
        consts = ctx.enter_context(tc.tile_pool(name="consts", bufs=1))
        persist = ctx.enter_context(tc.tile_pool(name="persist", bufs=1))

        # constants / weights
        wqa = consts.tile([128, 32], BF16, tag="wqa")
        wqb = consts.tile([128, 32], BF16, tag="wqb")
        nc.sync.dma_start(wqa, wqkT[0:128, :])
        nc.sync.dma_start(wqb, wqkT[128:256, :])
        wva = consts.tile([128, 130], BF16, tag="wva")
        wvb = consts.tile([128, 130], BF16, tag="wvb")
        nc.sync.dma_start(wva, wvT[0:128, :])
        nc.sync.dma_start(wvb, wvT[128:256, :])
        qkb = consts.tile([32, 1], F32, tag="qkb")
        nc.sync.dma_start(qkb, qk_bias[:, :])
        vbias = consts.tile([128, 1, 130], BF16, tag="vbias")
        nc.sync.dma_start(vbias[:, 0, :], vbias_bc[:, :])
        identb = consts.tile([128, 128], BF16, tag="identb")
        make_identity(nc, identb)

        # persistent activations
        # band-packed operand stores: partition 32*(h%4)+c, c<8
        q_sb = persist.tile([128, 2, H // 4, W], BF16, tag="q")    # 16 KiB
        k_sb = persist.tile([128, 2, H // 4, W], BF16, tag="k")    # 16 KiB
        qc_sb = persist.tile([128, 2, W // 4, H], BF16, tag="qc")  # 16 KiB
        kc_sb = persist.tile([128, 2, W // 4, H], BF16, tag="kc")  # 16 KiB
        # pixel-major value stores, channel innermost
        vT_sb = persist.tile([128, H, 130], BF16, tag="vT")        # 32.5 KiB
        vTc_sb = persist.tile([128, W, 130], BF16, tag="vTc")      # 32.5 KiB

        # vTc ones-columns (softmax denominator): constant, no transpose
        nc.vector.memset(vTc_sb[:, :, 64:65], 1.0)
        nc.vector.memset(vTc_sb[:, :, 129:130], 1.0)

        # ---------------- Phase B: projections ----------------
        with (
            tc.tile_pool(name="qkflat", bufs=1) as flatpool,
            tc.tile_pool(name="xchunk", bufs=2) as xpool,
            tc.tile_pool(name="pq", bufs=2, space="PSUM") as pqpool,
            tc.tile_pool(name="pv", bufs=4, space="PSUM") as pvpool,
        ):
            fr = flatpool.tile([32, PIX], BF16, tag="fr")  # [c, h*128+w]
            fc = flatpool.tile([32, PIX], BF16, tag="fc")  # [c, w*128+h]

            def band_qk(hb0, nhb, src, qdst, kdst, eng):
                # src: flat [32, pix] view c (hb b inner); move to band-packed
                sv = src[:, :].rearrange("c (hb b w) -> c b hb w", b=4, w=128)
                for bb in range(4):
                    for hh in range(2):
                        eng.dma_start(
                            qdst[32 * bb : 32 * bb + 8, hh, hb0 : hb0 + nhb, :],
                            sv[8 * hh : 8 * hh + 8, bb, hb0 : hb0 + nhb, :])
                        eng.dma_start(
                            kdst[32 * bb : 32 * bb + 8, hh, hb0 : hb0 + nhb, :],
                            sv[16 + 8 * hh : 24 + 8 * hh, bb, hb0 : hb0 + nhb, :])

            CHUNK = 512  # pixels per chunk = 4 rows
            NCH = PIX // CHUNK
            for chi in range(NCH):
                c0 = chi * CHUNK
                r0 = c0 // 128
                eng = nc.sync if chi % 2 == 0 else nc.scalar
                xa = xpool.tile([128, CHUNK], F32, tag="xa")
                xb = xpool.tile([128, CHUNK], F32, tag="xb")
                eng.dma_start(xa, x_in[0:128, c0 : c0 + CHUNK])
                eng.dma_start(xb, x_in[128:256, c0 : c0 + CHUNK])
                # bf16 copies: cheaper LDWEIGHTS (FWL) for the matmuls;
                # split the cast between DVE and ACT
                xab = xpool.tile([128, CHUNK], BF16, tag="xab")
                xbb = xpool.tile([128, CHUNK], BF16, tag="xbb")
                nc.vector.tensor_copy(xab, xa[:, :])
                nc.scalar.activation(xbb, xb[:, :], AF.Copy)
                xav = xab[:, :].rearrange("p (r w) -> p r w", w=128)
                xbv = xbb[:, :].rearrange("p (r w) -> p r w", w=128)

                # qk projection, row-pixel order; bias fused in ACT evac
                pq = pqpool.tile([32, 512], F32, tag="pq")
                nc.tensor.matmul(pq, wqa, xab[:, :], start=True, stop=False)
                nc.tensor.matmul(pq, wqb, xbb[:, :], start=False, stop=True)
                nc.scalar.activation(fr[:, c0 : c0 + CHUNK], pq,
                                     AF.Identity, bias=qkb)

                # vT projection: 2 rows per PSUM half-bank tile; bias folded
                # into the DVE evacuation (vbias broadcast over rows)
                for s2 in range(2):
                    pv = pvpool.tile([128, 2, 130], F32, tag="pv")
                    for s3 in range(2):
                        srow = 2 * s2 + s3
                        # start=True only on the bank's first matmul: its
                        # has_written clear is bank-wide
                        nc.tensor.matmul(pv[:, s3, :], xav[:, srow, :], wva,
                                         start=(s3 == 0), stop=False,
                                         skip_group_check=True)
                        nc.tensor.matmul(pv[:, s3, :], xbv[:, srow, :], wvb,
                                         start=False, stop=(s3 == 1),
                                         skip_group_check=True)
                    nc.vector.tensor_tensor(
                        vT_sb[:, r0 + 2 * s2 : r0 + 2 * s2 + 2, :], pv,
                        vbias.to_broadcast((128, 2, 130)),
                        mybir.AluOpType.add)

                # col-major flat store slices: fc[:, :, h-slice] only needs
                # fr rows h-slice -> overlap the permute with projection
                if chi % 8 == 7:
                    hs = (chi // 8) * 32
                    frv = fr[:, :].rearrange("c (h w) -> c w h", w=W)
                    fcv = fc[:, :].rearrange("c (w h) -> c w h", h=H)
                    nc.vector.tensor_copy(fcv[:, :, hs : hs + 32],
                                          frv[:, :, hs : hs + 32])
                    # q/k banding for the finished 8 hb slices (32 rows):
                    # SBUF->SBUF partition-move DMAs, overlapped with B
                    band_qk((chi // 8) * 8, 8, fr, q_sb, k_sb, nc.gpsimd)

            # qc/kc banding needs the full fc; issue now so it overlaps the
            # vTc transpose phase below (DMA queues are idle there).
            # Descriptor generation is on the triggering engine -- spread the
            # 16 dma_starts across four engines so it isn't serialized.
            sv_c = fc[:, :].rearrange("c (hb b w) -> c b hb w", b=4, w=128)
            engs = [nc.gpsimd, nc.sync, nc.scalar]
            for bb in range(4):
                for hh in range(2):
                    eng = engs[(2 * bb + hh) % 3]
                    eng.dma_start(
                        qc_sb[32 * bb : 32 * bb + 8, hh, :, :],
                        sv_c[8 * hh : 8 * hh + 8, bb, :, :])
                    eng.dma_start(
                        kc_sb[32 * bb : 32 * bb + 8, hh, :, :],
                        sv_c[16 + 8 * hh : 24 + 8 * hh, bb, :, :])

        # ---------------- Phase B2: vTc via PE transposes ----------------
        # vT[w, h, c] -> vTc[h, w, c]; per channel, batched 4 per bank.
        # channels 64/129 are constant ones (memset above): 128 real ones.
        REAL_CH = [c for c in range(130) if c != 64 and c != 129]
        with tc.tile_pool(name="ptr", bufs=2, space="PSUM") as ptrpool:
            for cb in range(32):
                ptr = ptrpool.tile([128, 4, 128], BF16, tag="ptr")
                chs = REAL_CH[cb * 4 : cb * 4 + 4]
                for cj, cch in enumerate(chs):
                    nc.tensor.matmul(ptr[:, cj, :], vT_sb[:, :, cch], identb,
                                     start=True, stop=True, is_transpose=True)
                if cb % 2 == 0:
                    nc.vector.tensor_copy(
                        vTc_sb[:, :, chs[0] : chs[0] + 4],
                        ptr[:, 0:4, :].rearrange("p c w -> p w c"))
                else:
                    nc.scalar.copy(
                        vTc_sb[:, :, chs[0] : chs[0] + 4],
                        ptr[:, 0:4, :].rearrange("p c w -> p w c"))

        # ---------------- Phase C: attention ----------------
        # One f32 [128, 4, 512] PSUM tile (4 banks; bank j = group row j)
        # per (g, d) iteration:
        #   [:, j, 0:128]    energy hh=0, later (d=1 only) the output
        #                    transpose as bf16 (bitcast, 64 f32 slots)
        #   [:, j, 128:256]  energy hh=1
        #   [:, j, 256:386]  AV out: [256:320] ch hh0, [320] den hh0,
        #                    [321:385] ch hh1, [385] den hh1
        with (
            tc.tile_pool(name="mega", bufs=2, space="PSUM") as megapool,
            tc.tile_pool(name="pt", bufs=3) as ptpool,
            tc.tile_pool(name="tt", bufs=5) as tpool,
            tc.tile_pool(name="au", bufs=2) as aupool,
            tc.tile_pool(name="rc", bufs=4) as rcpool,
            tc.tile_pool(name="io", bufs=4) as iopool,
        ):
            def emit_tail_front(pg, t0, t1, pat_mega):
                # deferred combine/transpose for group pg; the transpose
                # lands in [386:450] (bf16) of the CURRENT group's d0 mega
                # buffer -- disjoint from its energy/po regions.
                eng = nc.sync if pg % 2 == 0 else nc.scalar
                xres = iopool.tile([128, G, 128], F32, tag="xres")
                eng.dma_start(
                    xres[:, :, :].rearrange("p g w -> p (g w)"),
                    x_res[:, pg * 512 : (pg + 1) * 512])
                au = aupool.tile([128, G, 128], BF16, tag="au")
                nc.gpsimd.tensor_tensor(
                    au,
                    t0[:, :, :, :].rearrange("p j h c -> p j (h c)"),
                    t1[:, :, :, :].rearrange("p j h c -> p j (h c)"),
                    mybir.AluOpType.add)
                for j in range(G):
                    nc.tensor.matmul(
                        pat_mega[:, j, 386:450].bitcast(BF16),
                        au[:, j, :], identb,
                        start=False, stop=True, is_transpose=True,
                        skip_group_check=True)
                return xres

            def emit_tail_back(pg, xres, pat_mega):
                # residual add + store, issued after the current group's
                # mults so it can't head-of-line-block the DVE queue
                eng = nc.sync if pg % 2 == 0 else nc.scalar
                res = iopool.tile([128, G, 128], F32, tag="res")
                nc.vector.tensor_tensor(
                    res, pat_mega[:, :, 386:450].bitcast(BF16),
                    xres, mybir.AluOpType.add)
                eng.dma_start(out[:, pg * 512 : (pg + 1) * 512],
                              res[:, :, :].rearrange("p g w -> p (g w)"))

            prev = None  # (g-1, til_d0, til_d1)
            for g in range(NG):
                megas = []
                pTs = []
                for d in range(2):
                    ks = k_sb if d == 0 else kc_sb
                    qs = q_sb if d == 0 else qc_sb
                    mega = megapool.tile([128, 4, 512], F32, tag="mega")
                    megas.append(mega)
                    # energies: 4 concurrent row-group matmuls per hh,
                    # hh=1 lands at +128 in the same bank (start only on
                    # the bank's first write -- the clear is bank-wide)
                    for hh in range(2):
                        for j in range(G):
                            nc.tensor.matmul(
                                mega[:, j, 128 * hh : 128 * hh + 128],
                                ks[32 * j : 32 * j + 8, hh, g, :],
                                qs[32 * j : 32 * j + 8, hh, g, :],
                                start=(hh == 0), stop=(hh == 1),
                                tile_position=(32 * j, 0),
                                skip_group_check=True,
                            )
                    # one EXP over all 4 banks x both heads (FD=1024)
                    pT = ptpool.tile([128, 4, 2, 128], BF16, tag="pt")
                    nc.scalar.activation(
                        pT,
                        mega[:, :, 0:256].rearrange(
                            "p j (h w) -> p j h w", w=128),
                        AF.Exp)
                    pTs.append(pT)
                # previous group's tail: au waits on GpSimd work issued
                # last g; the PE transposes sit here, after this group's
                # energies, so the g-boundary chain stays off the PE queue
                if prev is not None:
                    pxres = emit_tail_front(prev[0], prev[1], prev[2],
                                            megas[0])
                t_dir = []
                for d in range(2):
                    vs = vT_sb if d == 0 else vTc_sb
                    mega = megas[d]
                    # AV: po[w, ch]+den into [256:386] of each bank
                    for hh in range(2):
                        for j in range(G):
                            i = g * G + j
                            nc.tensor.matmul(
                                mega[:, j, 256 + 65 * hh : 321 + 65 * hh],
                                pTs[d][:, j, hh, :],
                                vs[:, i, 65 * hh : 65 * hh + 65],
                                start=False, stop=True,
                                skip_group_check=True,
                            )
                    pov = mega[:, :, 256:386].rearrange(
                        "p j (h c) -> p j h c", c=65)
                    rec = rcpool.tile([128, 4, 2, 1], F32, tag="rc")
                    nc.vector.reciprocal(rec, pov[:, :, :, 64:65])
                    til = tpool.tile([128, 4, 2, CV], BF16, tag="t")
                    nc.vector.tensor_tensor(
                        til, pov[:, :, :, 0:64],
                        rec.to_broadcast((128, 4, 2, CV)),
                        mybir.AluOpType.mult,
                    )
                    t_dir.append(til)
                if prev is not None:
                    emit_tail_back(prev[0], pxres, megas[0])
                prev = (g, t_dir[0], t_dir[1])
            # flush the last group's tail into a fresh rotation buffer
            mega_fl = megapool.tile([128, 4, 512], F32, tag="mega")
            fxres = emit_tail_front(prev[0], prev[1], prev[2], mega_fl)
            emit_tail_back(prev[0], fxres, mega_fl)

    return nc


def mega_pat_view(mega, j):
    # bank j, f32 slots [0:64] viewed as [128, 128] bf16
    return mega[:, j, 0:64].bitcast(BF16)


def _prep_core_inputs(core, x, Wq, bq, Wk, bk, Wv, bv, gamma):
    b = core // 2
    p = core % 2
    g = float(np.asarray(gamma).reshape(-1)[0])
    qsl = slice(16 * p, 16 * p + 16)
    vsl = slice(128 * p, 128 * p + 128)

    import ml_dtypes
    bf = ml_dtypes.bfloat16

    wqk = np.zeros((C, 32), np.float32)
    wqk[:, 0:16] = Wq[qsl].T       # q head even(8) | q head odd(8)
    wqk[:, 16:32] = Wk[qsl].T
    wqk = wqk.astype(bf)
    qkb = np.concatenate([bq[qsl], bk[qsl]]).reshape(32, 1).astype(np.float32)

    wv_eff = (g * Wv[vsl]).astype(np.float32)     # [128, 256]
    bv_eff = (g * bv[vsl]).astype(np.float32)
    wvt = np.zeros((C, 130), np.float32)
    wvt[:, 0:64] = wv_eff[0:64].T
    wvt[:, 65:129] = wv_eff[64:128].T
    wvt = wvt.astype(bf)
    vbias = np.zeros((1, 130), np.float32)
    vbias[0, 0:64] = bv_eff[0:64]
    vbias[0, 64] = 1.0
    vbias[0, 65:129] = bv_eff[64:128]
    vbias[0, 129] = 1.0
    vbias_bc = np.broadcast_to(vbias, (128, 130)).astype(bf)

    return {
        "x_in": np.ascontiguousarray(x[b].reshape(C, PIX), np.float32),
        "x_res": np.ascontiguousarray(x[b, vsl].reshape(128, PIX), np.float32),
        "wqkT": wqk,
        "qk_bias": qkb,
        "wvT": wvt,
        "vbias_bc": np.ascontiguousarray(vbias_bc),
    }


_NC_CACHE = None


def _get_nc():
    global _NC_CACHE
    if _NC_CACHE is None:
        nc = build_program()
        nc.compile()
        _NC_CACHE = nc
    return _NC_CACHE


def kernel(x, Wq, bq, Wk, bk, Wv, bv, gamma, _trace=False, _trace_kwargs=None):
    from concourse.bass_utils import run_bass_kernel_spmd

    nc = _get_nc()
    in_maps = [
        _prep_core_inputs(core, x, Wq, bq, Wk, bk, Wv, bv, gamma)
        for core in range(NCORES)
    ]
    res = run_bass_kernel_spmd(
        nc, in_maps, list(range(NCORES)), trace=_trace,
        **(_trace_kwargs or {}),
    )
    outp = np.empty((B, C, H, W), np.float32)
    for core in range(NCORES):
        b, p = core // 2, core % 2
        outp[b, 128 * p : 128 * p + 128] = (
            res.results[core]["out"].reshape(128, H, W)
        )
    if _trace:
        kernel.last_results = res
    return outp

